# revision 6
# baseline (speedup 1.0000x reference)
"""Trainium2 Bass kernel for moe_routing bilinear gate.

out = sigmoid(q^T W0 r + q^T A[hop] B[hop]^T r + sum(v*q*r) + b[hop])

Sharding: pure data parallel over batch across 8 cores. Params replicated.

Per-core pipeline (feature-major, fp16 on-chip):
  - SWDGE cast-load q,r fp32->fp16 (natural layout, 128-row blocks)
  - PE transposes -> qT,rT [d, samples]
  - matmuls: u = (W0+diag(v))^T-applied, qa = A_cat proj, rb = B_cat proj
  - prod2 = u * rT  (base + hadamard terms)
  - qam = (hop==gid) * qa  (fused mask select), mab = qam * rbS
  - reduction matmuls (ones-column lhsT) accumulate 32 tiles into PSUM [32,512]
  - bias b[hop] via degree-4 Horner polynomial of hop, add, sigmoid, store
"""

import os
import sys
from contextlib import ExitStack

import numpy as np

if "/opt/trn_rl_repo" not in sys.path:
    sys.path.insert(0, "/opt/trn_rl_repo")

import concourse.bass as bass  # noqa: E402
import concourse.bacc as bacc  # noqa: E402
import concourse.tile as tile  # noqa: E402
from concourse import mybir  # noqa: E402
from concourse.bass_utils import run_bass_kernel_spmd  # noqa: E402

B_SZ, D, RHO, L = 1048576, 128, 8, 4
NCORES = 8
N = B_SZ // NCORES  # 131072 samples per core

P = 128
TS = 512            # samples per tile (one PSUM bank of fp32)
PAIR = 1024         # samples per pair (DVE/ACT op batching)
NPAIR = N // PAIR   # 128
FILL_PAIRS = 16     # pairs per output fill (32 tiles -> PSUM [32, 512])
FILL = FILL_PAIRS * PAIR  # 16384 samples
NFILL = N // FILL   # 8

F16 = mybir.dt.float16
F32 = mybir.dt.float32
I32 = mybir.dt.int32
ALU = mybir.AluOpType
ACTF = mybir.ActivationFunctionType

_CACHE = {}


def _emit(ctx, tc, io, bcoef, n):
    nc = tc.nc
    npair = n // PAIR
    nfill = max(1, n // FILL)
    fill_pairs = npair // nfill
    fill = fill_pairs * PAIR
    ntile_fill = 2 * fill_pairs  # tiles per fill (<= 32)
    q, r, hop, o, wp, ac, bc, idm, sel, gid = io
    c4, c3, c2, c1, c0 = [float(x) for x in bcoef]

    const = ctx.enter_context(tc.tile_pool(name="const", bufs=1))
    wp_s = const.tile([P, P], F16, tag="wp")
    nc.sync.dma_start(wp_s[:], wp)
    ac_s = const.tile([P, 40], F16, tag="ac")
    nc.sync.dma_start(ac_s[:], ac)
    bc_s = const.tile([P, 40], F16, tag="bc")
    nc.sync.dma_start(bc_s[:], bc)
    id_s = const.tile([P, P], F16, tag="idm")
    nc.sync.dma_start(id_s[:], idm)
    sel_s = const.tile([P, 63], F16, tag="sel")
    nc.sync.dma_start(sel_s[:], sel)
    gid_s = const.tile([40, 1], F16, tag="gid")
    nc.sync.dma_start(gid_s[:], gid)

    # hop as fp16, [npair, PAIR]: partition pp holds hop[PAIR*pp : PAIR*(pp+1)]
    hop32 = const.tile([npair, PAIR], I32, tag="hop32")
    nc.sync.dma_start(hop32[:], hop.rearrange("(p f) -> p f", p=npair))
    hop16 = const.tile([npair, PAIR], F16, tag="hop16")
    nc.vector.tensor_copy(hop16[:], hop32[:])

    # pools
    qn_p = ctx.enter_context(tc.tile_pool(name="qn", bufs=3))
    rn_p = ctx.enter_context(tc.tile_pool(name="rn", bufs=3))
    qt_p = ctx.enter_context(tc.tile_pool(name="qt", bufs=2))
    rt_p = ctx.enter_context(tc.tile_pool(name="rt", bufs=2))
    hb_p = ctx.enter_context(tc.tile_pool(name="hb", bufs=2))
    px_p = ctx.enter_context(tc.tile_pool(name="px", bufs=2))
    rbs_p = ctx.enter_context(tc.tile_pool(name="rbs", bufs=2))
    qam_p = ctx.enter_context(tc.tile_pool(name="qam", bufs=2))
    mab_p = ctx.enter_context(tc.tile_pool(name="mab", bufs=2))
    fin_p = ctx.enter_context(tc.tile_pool(name="fin", bufs=2))

    tr_ps = ctx.enter_context(tc.tile_pool(name="trps", bufs=1, space="PSUM"))
    u_ps = ctx.enter_context(tc.tile_pool(name="ups", bufs=1, space="PSUM"))
    qa_ps = ctx.enter_context(tc.tile_pool(name="qaps", bufs=1, space="PSUM"))
    rb_ps = ctx.enter_context(tc.tile_pool(name="rbps", bufs=1, space="PSUM"))
    out_ps = ctx.enter_context(tc.tile_pool(name="outps", bufs=1, space="PSUM"))

    out_acc = None
    for pp in range(npair):
        j0 = pp * PAIR
        # natural-layout loads with fp32->fp16 cast during DMA (SWDGE).
        # col block b (128 wide) holds rows j0+128*b .. j0+128*b+127.
        qn = qn_p.tile([P, PAIR], F16, tag="qn")
        nc.gpsimd.dma_start(
            qn[:].rearrange("p (b d) -> p b d", d=P),
            q[j0:j0 + PAIR, :].rearrange("(b p) d -> p b d", p=P),
        )
        rn = rn_p.tile([P, PAIR], F16, tag="rn")
        nc.gpsimd.dma_start(
            rn[:].rearrange("p (b d) -> p b d", d=P),
            r[j0:j0 + PAIR, :].rearrange("(b p) d -> p b d", p=P),
        )

        # PE transposes -> feature-major [d, sample]
        qt_psum = tr_ps.tile([P, PAIR], F16, tag="tr")
        for b in range(PAIR // P):
            nc.tensor.transpose(
                qt_psum[:, P * b:P * (b + 1)], qn[:, P * b:P * (b + 1)], id_s[:]
            )
        qt = qt_p.tile([P, PAIR], F16, tag="qt")
        nc.scalar.copy(qt[:], qt_psum[:])

        rt_psum = tr_ps.tile([P, PAIR], F16, tag="tr")
        for b in range(PAIR // P):
            nc.tensor.transpose(
                rt_psum[:, P * b:P * (b + 1)], rn[:, P * b:P * (b + 1)], id_s[:]
            )
        rt = rt_p.tile([P, PAIR], F16, tag="rt")
        nc.scalar.copy(rt[:], rt_psum[:])

        # hop broadcast across 40 partitions (SBUF->SBUF DMA, stride-0 source)
        hb = hb_p.tile([40, PAIR], F16, tag="hb")
        row = hop16[pp:pp + 1, :]
        row_b = bass.AP(
            row.tensor, row.offset,
            [list(row.ap[0]), [0, 40]] + [list(a) for a in row.ap[1:]],
        )
        nc.sync.dma_start(hb[:], row_b)

        # matmuls (N=512 each)
        u = u_ps.tile([P, PAIR], F32, tag="u")
        qa = qa_ps.tile([40, PAIR], F32, tag="qa")
        rb = rb_ps.tile([40, PAIR], F32, tag="rb")
        for s in range(2):
            sl = slice(TS * s, TS * (s + 1))
            nc.tensor.matmul(u[:, sl], wp_s[:], qt[:, sl], start=True, stop=True)
            nc.tensor.matmul(qa[:, sl], ac_s[:], qt[:, sl], start=True, stop=True)
            nc.tensor.matmul(rb[:, sl], bc_s[:], rt[:, sl], start=True, stop=True)

        # elementwise stage
        px = px_p.tile([P, PAIR], F16, tag="px")
        nc.vector.tensor_tensor(px[:], u[:], rt[:], ALU.mult)
        rbs = rbs_p.tile([40, PAIR], F16, tag="rbs")
        nc.scalar.copy(rbs[:], rb[:])
        qam = qam_p.tile([40, PAIR], F16, tag="qam")
        nc.vector.scalar_tensor_tensor(
            qam[:], hb[:], gid_s[:], qa[:], ALU.is_equal, ALU.mult
        )
        mab = mab_p.tile([40, PAIR], F16, tag="mab")
        nc.vector.tensor_tensor(mab[:], qam[:], rbs[:], ALU.mult)

        # reduction matmuls accumulate into out_acc [32, 512]
        if pp % fill_pairs == 0:
            out_acc = out_ps.tile([ntile_fill, TS], F32, tag="out")
        for s in range(2):
            tt = (2 * pp + s) % ntile_fill
            sl = slice(TS * s, TS * (s + 1))
            nc.tensor.matmul(
                out_acc[:], sel_s[:, 31 - tt:31 - tt + ntile_fill], px[:, sl],
                start=(tt == 0), stop=False, skip_group_check=True,
            )
            nc.tensor.matmul(
                out_acc[:], sel_s[0:40, 31 - tt:31 - tt + ntile_fill], mab[:, sl],
                start=False, stop=(tt == ntile_fill - 1), skip_group_check=True,
            )

        # final stage per fill
        if pp % fill_pairs == fill_pairs - 1:
            f = pp // fill_pairs
            hsm32 = fin_p.tile([ntile_fill, TS], I32, tag="hsm32")
            nc.sync.dma_start(
                hsm32[:],
                hop[f * fill:(f + 1) * fill].rearrange("(p j) -> p j", p=ntile_fill),
            )
            hf = fin_p.tile([ntile_fill, TS], F32, tag="hf")
            nc.vector.tensor_copy(hf[:], hsm32[:])
            # Horner: bias = ((((c4*h)+c3)*h+c2)*h+c1)*h+c0
            pt = fin_p.tile([ntile_fill, TS], F32, tag="pt")
            nc.vector.tensor_scalar(pt[:], hf[:], c4, c3, ALU.mult, ALU.add)
            for ck in (c2, c1, c0):
                tmp = fin_p.tile([ntile_fill, TS], F32, tag="tmp")
                nc.vector.scalar_tensor_tensor(
                    tmp[:], pt[:], 0.0, hf[:], ALU.bypass, ALU.mult
                )
                pt = fin_p.tile([ntile_fill, TS], F32, tag="pt")
                nc.vector.tensor_scalar_add(pt[:], tmp[:], ck)
            logit = fin_p.tile([ntile_fill, TS], F32, tag="logit")
            nc.vector.tensor_tensor(logit[:], out_acc[:], pt[:], ALU.add)
            osb = fin_p.tile([ntile_fill, TS], F32, tag="osb")
            nc.scalar.activation(osb[:], logit[:], ACTF.Sigmoid)
            nc.sync.dma_start(
                o[f * fill:(f + 1) * fill].rearrange("(p j) -> p j", p=ntile_fill), osb[:]
            )


def _build(bcoef, n=N):
    key = (n,) + tuple(np.asarray(bcoef, dtype=np.float64).tolist())
    if key in _CACHE:
        return _CACHE[key]
    nc = bacc.Bacc("TRN2", target_bir_lowering=False, debug=False)
    q = nc.dram_tensor("q", [n, D], F32, kind="ExternalInput").ap()
    r = nc.dram_tensor("r", [n, D], F32, kind="ExternalInput").ap()
    hop = nc.dram_tensor("hop", [n], I32, kind="ExternalInput").ap()
    o = nc.dram_tensor("o", [n], F32, kind="ExternalOutput").ap()
    wp = nc.dram_tensor("wp", [P, P], F16, kind="ExternalInput").ap()
    ac = nc.dram_tensor("ac", [P, 40], F16, kind="ExternalInput").ap()
    bc = nc.dram_tensor("bc", [P, 40], F16, kind="ExternalInput").ap()
    idm = nc.dram_tensor("idm", [P, P], F16, kind="ExternalInput").ap()
    sel = nc.dram_tensor("sel", [P, 63], F16, kind="ExternalInput").ap()
    gid = nc.dram_tensor("gid", [40, 1], F16, kind="ExternalInput").ap()
    io = (q, r, hop, o, wp, ac, bc, idm, sel, gid)
    with tile.TileContext(nc) as tc, ExitStack() as ctx:
        _emit(ctx, tc, io, bcoef, n)
    nc.compile()
    _CACHE[key] = nc
    return nc


def _prep(q, r, hop, W0, A, Bm, v, b):
    q = np.asarray(q, dtype=np.float32)
    r = np.asarray(r, dtype=np.float32)
    hop = np.asarray(hop)
    if hop.dtype != np.int32:
        hop = hop.astype(np.int32)
    W0 = np.asarray(W0, dtype=np.float32)
    A = np.asarray(A, dtype=np.float32)
    Bm = np.asarray(Bm, dtype=np.float32)
    v = np.asarray(v, dtype=np.float32)
    b = np.asarray(b, dtype=np.float64)

    wp = (W0[0] + np.diag(v)).astype(np.float16)
    ac = A.transpose(1, 0, 2).reshape(D, (L + 1) * RHO).astype(np.float16)
    bc = Bm.transpose(1, 0, 2).reshape(D, (L + 1) * RHO).astype(np.float16)
    idm = np.eye(P, dtype=np.float16)
    sel = np.zeros((P, 63), dtype=np.float16)
    sel[:, 31] = 1.0
    gid = (np.arange((L + 1) * RHO) // RHO).reshape(-1, 1).astype(np.float16)
    bcoef = np.polyfit(np.arange(L + 1, dtype=np.float64), b, L)

    consts = dict(wp=wp, ac=ac, bc=bc, idm=idm, sel=sel, gid=gid)
    in_maps = []
    for c in range(NCORES):
        sl = slice(c * N, (c + 1) * N)
        in_maps.append(
            dict(q=q[sl], r=r[sl], hop=hop[sl], **consts)
        )
    return in_maps, bcoef


def _run(inputs, trace=False, tmpdir=None):
    in_maps, bcoef = _prep(**inputs)
    nc = _build(bcoef)
    res = run_bass_kernel_spmd(
        nc, in_maps, list(range(NCORES)), trace=trace, tmpdir=tmpdir
    )
    out = np.concatenate([np.asarray(res.results[c]["o"]) for c in range(NCORES)])
    return out, res


def kernel(**inputs):
    out, _ = _run(inputs)
    return out


# revision 17
# speedup vs baseline: 1.5380x; 1.5380x over previous
"""Trainium2 Bass kernel for moe_routing bilinear gate.

out = sigmoid(q^T W0 r + q^T A[hop] B[hop]^T r + sum(v*q*r) + b[hop])

Sharding: pure data parallel over batch across 8 cores. Params replicated.

Per-core pipeline (feature-major, fp16 on-chip):
  - SWDGE cast-load q,r fp32->fp16 (natural layout, 128-row blocks)
  - PE transposes -> qT,rT [d, samples]
  - matmuls: u = (W0+diag(v))^T-applied, qa = A_cat proj, rb = B_cat proj
  - prod2 = u * rT  (base + hadamard terms)
  - qam = (hop==gid) * qa  (fused mask select), mab = qam * rbS
  - reduction matmuls (ones-column lhsT) accumulate 32 tiles into PSUM [32,512]
  - bias b[hop] via degree-4 Horner polynomial of hop, add, sigmoid, store
"""

import os
import sys
from contextlib import ExitStack

import numpy as np

if "/opt/trn_rl_repo" not in sys.path:
    sys.path.insert(0, "/opt/trn_rl_repo")

import concourse.bass as bass  # noqa: E402
import concourse.bacc as bacc  # noqa: E402
import concourse.tile as tile  # noqa: E402
from concourse import mybir  # noqa: E402
from concourse.bass_utils import run_bass_kernel_spmd  # noqa: E402

B_SZ, D, RHO, L = 1048576, 128, 8, 4
NCORES = 8
N = B_SZ // NCORES  # 131072 samples per core

P = 128
TS = 512            # samples per tile (one PSUM bank of fp32)
PAIR = 1024         # samples per pair (DVE/ACT op batching)
NPAIR = N // PAIR   # 128
FILL_PAIRS = 16     # pairs per output fill (32 tiles -> PSUM [32, 512])
FILL = FILL_PAIRS * PAIR  # 16384 samples
NFILL = N // FILL   # 8

F16 = mybir.dt.float16
F32 = mybir.dt.float32
I32 = mybir.dt.int32
ALU = mybir.AluOpType
ACTF = mybir.ActivationFunctionType

_CACHE = {}


def _emit(ctx, tc, io, bcoef, n):
    nc = tc.nc
    npair = n // PAIR
    nfill = max(1, n // FILL)
    fill_pairs = npair // nfill
    fill = fill_pairs * PAIR
    ntile_fill = 2 * fill_pairs  # tiles per fill (<= 32)
    q, r, hop, o, wp, ac, bc, idm, sel, gid = io
    c4, c3, c2, c1, c0 = [float(x) for x in bcoef]

    const = ctx.enter_context(tc.tile_pool(name="const", bufs=1))
    wp_s = const.tile([P, P], F16, tag="wp")
    nc.sync.dma_start(wp_s[:], wp)
    ac_s = const.tile([P, 40], F16, tag="ac")
    nc.sync.dma_start(ac_s[:], ac)
    bc_s = const.tile([P, 40], F16, tag="bc")
    nc.sync.dma_start(bc_s[:], bc)
    id_s = const.tile([P, P], F16, tag="idm")
    nc.sync.dma_start(id_s[:], idm)
    sel_s = const.tile([P, 63], F16, tag="sel")
    nc.sync.dma_start(sel_s[:], sel)
    gid_s = const.tile([40, 1], F16, tag="gid")
    nc.sync.dma_start(gid_s[:], gid)

    # hop as fp16, [npair, PAIR]: partition pp holds hop[PAIR*pp : PAIR*(pp+1)]
    hop32 = const.tile([npair, PAIR], I32, tag="hop32")
    nc.sync.dma_start(hop32[:], hop.rearrange("(p f) -> p f", p=npair))
    hop16 = const.tile([npair, PAIR], F16, tag="hop16")
    nc.vector.tensor_copy(hop16[:], hop32[:])

    # pools
    qn32_p = ctx.enter_context(tc.tile_pool(name="qn32", bufs=3))
    rn32_p = ctx.enter_context(tc.tile_pool(name="rn32", bufs=3))
    qn_p = ctx.enter_context(tc.tile_pool(name="qn", bufs=3))
    rn_p = ctx.enter_context(tc.tile_pool(name="rn", bufs=3))
    qt_p = ctx.enter_context(tc.tile_pool(name="qt", bufs=3))
    rt_p = ctx.enter_context(tc.tile_pool(name="rt", bufs=3))
    hb_p = ctx.enter_context(tc.tile_pool(name="hb", bufs=3))
    px_p = ctx.enter_context(tc.tile_pool(name="px", bufs=3))
    rbs_p = ctx.enter_context(tc.tile_pool(name="rbs", bufs=4))
    qam_p = ctx.enter_context(tc.tile_pool(name="qam", bufs=4))
    mab_p = ctx.enter_context(tc.tile_pool(name="mab", bufs=4))
    fin_p = ctx.enter_context(tc.tile_pool(name="fin", bufs=2))

    trq_ps = ctx.enter_context(tc.tile_pool(name="trqps", bufs=1, space="PSUM"))
    trr_ps = ctx.enter_context(tc.tile_pool(name="trrps", bufs=1, space="PSUM"))
    u_ps = ctx.enter_context(tc.tile_pool(name="ups", bufs=3, space="PSUM"))
    qa_ps = ctx.enter_context(tc.tile_pool(name="qaps", bufs=1, space="PSUM"))
    rb_ps = ctx.enter_context(tc.tile_pool(name="rbps", bufs=1, space="PSUM"))
    out_ps = ctx.enter_context(tc.tile_pool(name="outps", bufs=1, space="PSUM"))

    out_acc = None
    for pp in range(npair):
        j0 = pp * PAIR
        # natural-layout loads with fp32->fp16 cast during DMA (SWDGE).
        # col block b (128 wide) holds rows j0+128*b .. j0+128*b+127.
        qn32 = qn32_p.tile([P, PAIR], F32, tag="qn32")
        nc.sync.dma_start(
            qn32[:].rearrange("p (b d) -> p b d", d=P),
            q[j0:j0 + PAIR, :].rearrange("(b p) d -> p b d", p=P),
        )
        rn32 = rn32_p.tile([P, PAIR], F32, tag="rn32")
        nc.sync.dma_start(
            rn32[:].rearrange("p (b d) -> p b d", d=P),
            r[j0:j0 + PAIR, :].rearrange("(b p) d -> p b d", p=P),
        )
        qn = qn_p.tile([P, PAIR], F16, tag="qn")
        nc.gpsimd.tensor_copy(qn[:], qn32[:])
        rn = rn_p.tile([P, PAIR], F16, tag="rn")
        nc.gpsimd.tensor_copy(rn[:], rn32[:])

        # PE transposes -> feature-major [d, sample]
        qt_psum = trq_ps.tile([P, PAIR], F16, tag="trq")
        for b in range(PAIR // P):
            nc.tensor.transpose(
                qt_psum[:, P * b:P * (b + 1)], qn[:, P * b:P * (b + 1)], id_s[:]
            )
        qt = qt_p.tile([P, PAIR], F16, tag="qt")
        nc.scalar.copy(qt[:], qt_psum[:])

        rt_psum = trr_ps.tile([P, PAIR], F16, tag="trr")
        for b in range(PAIR // P):
            nc.tensor.transpose(
                rt_psum[:, P * b:P * (b + 1)], rn[:, P * b:P * (b + 1)], id_s[:]
            )
        rt = rt_p.tile([P, PAIR], F16, tag="rt")
        nc.scalar.copy(rt[:], rt_psum[:])

        # hop broadcast across 40 partitions (SBUF->SBUF DMA, stride-0 source)
        hb = hb_p.tile([40, PAIR], F16, tag="hb")
        row = hop16[pp:pp + 1, :]
        row_b = bass.AP(
            row.tensor, row.offset,
            [list(row.ap[0]), [0, 40]] + [list(a) for a in row.ap[1:]],
        )
        nc.scalar.dma_start(hb[:], row_b)

        # matmuls + elementwise; row tt = (pp % fill_pairs) + fill_pairs*s
        if pp % fill_pairs == 0:
            out_acc = out_ps.tile([ntile_fill, TS], F32, tag="out")
        px = px_p.tile([P, PAIR], F16, tag="px")
        for s in range(2):
            sl = slice(TS * s, TS * (s + 1))
            u = u_ps.tile([P, TS], F32, tag="u")
            nc.tensor.matmul(u[:], wp_s[:], qt[:, sl], start=True, stop=True)
            nc.vector.tensor_tensor(px[:, sl], u[:], rt[:, sl], ALU.mult)
        for s in range(2):
            sl = slice(TS * s, TS * (s + 1))
            tt = (pp % fill_pairs) + fill_pairs * s
            qa = qa_ps.tile([40, TS], F32, tag="qa")
            nc.tensor.matmul(qa[:], ac_s[:], qt[:, sl], start=True, stop=True)
            rb = rb_ps.tile([40, TS], F32, tag="rb")
            nc.tensor.matmul(rb[:], bc_s[:], rt[:, sl], start=True, stop=True)
            rbs = rbs_p.tile([40, TS], F16, tag="rbs")
            nc.scalar.copy(rbs[:], rb[:])
            qam = qam_p.tile([40, TS], F16, tag="qam")
            nc.vector.scalar_tensor_tensor(
                qam[:], hb[:, sl], gid_s[:], qa[:], ALU.is_equal, ALU.mult
            )
            mab = mab_p.tile([40, TS], F16, tag="mab")
            nc.vector.tensor_tensor(mab[:], qam[:], rbs[:], ALU.mult)
            nc.tensor.matmul(
                out_acc[:], sel_s[:, 31 - tt:31 - tt + ntile_fill], px[:, sl],
                start=(tt == 0), stop=False, skip_group_check=True,
            )
            nc.tensor.matmul(
                out_acc[:], sel_s[0:40, 31 - tt:31 - tt + ntile_fill], mab[:],
                start=False, stop=(tt == ntile_fill - 1), skip_group_check=True,
            )

        # final stage per fill
        if pp % fill_pairs == fill_pairs - 1:
            f = pp // fill_pairs
            hsm = fin_p.tile([ntile_fill, TS], F16, tag="hsm")
            for s in range(2):
                nc.scalar.dma_start(
                    hsm[fill_pairs * s:fill_pairs * (s + 1), :],
                    hop16[fill_pairs * f:fill_pairs * (f + 1),
                          TS * s:TS * (s + 1)],
                )
            # Horner: bias = ((((c4*h)+c3)*h+c2)*h+c1)*h+c0
            pt = fin_p.tile([ntile_fill, TS], F32, tag="pt")
            nc.vector.tensor_scalar(pt[:], hsm[:], c4, c3, ALU.mult, ALU.add)
            for ck in (c2, c1, c0):
                tmp = fin_p.tile([ntile_fill, TS], F32, tag="tmp")
                nc.vector.scalar_tensor_tensor(
                    tmp[:], pt[:], 0.0, hsm[:], ALU.bypass, ALU.mult
                )
                pt = fin_p.tile([ntile_fill, TS], F32, tag="pt")
                nc.vector.tensor_scalar_add(pt[:], tmp[:], ck)
            logit = fin_p.tile([ntile_fill, TS], F32, tag="logit")
            nc.vector.tensor_tensor(logit[:], out_acc[:], pt[:], ALU.add)
            osb = fin_p.tile([ntile_fill, TS], F32, tag="osb")
            nc.scalar.activation(osb[:], logit[:], ACTF.Sigmoid)
            ov = o.rearrange("(ff qq s2 j) -> ff qq s2 j",
                             qq=fill_pairs, s2=2, j=TS)
            for s in range(2):
                nc.scalar.dma_start(
                    ov[f, :, s, :],
                    osb[fill_pairs * s:fill_pairs * (s + 1), :],
                )


def _build(bcoef, n=N):
    key = (n,) + tuple(np.asarray(bcoef, dtype=np.float64).tolist())
    if key in _CACHE:
        return _CACHE[key]
    nc = bacc.Bacc("TRN2", target_bir_lowering=False, debug=False)
    q = nc.dram_tensor("q", [n, D], F32, kind="ExternalInput").ap()
    r = nc.dram_tensor("r", [n, D], F32, kind="ExternalInput").ap()
    hop = nc.dram_tensor("hop", [n], I32, kind="ExternalInput").ap()
    o = nc.dram_tensor("o", [n], F32, kind="ExternalOutput").ap()
    wp = nc.dram_tensor("wp", [P, P], F16, kind="ExternalInput").ap()
    ac = nc.dram_tensor("ac", [P, 40], F16, kind="ExternalInput").ap()
    bc = nc.dram_tensor("bc", [P, 40], F16, kind="ExternalInput").ap()
    idm = nc.dram_tensor("idm", [P, P], F16, kind="ExternalInput").ap()
    sel = nc.dram_tensor("sel", [P, 63], F16, kind="ExternalInput").ap()
    gid = nc.dram_tensor("gid", [40, 1], F16, kind="ExternalInput").ap()
    io = (q, r, hop, o, wp, ac, bc, idm, sel, gid)
    with tile.TileContext(nc) as tc, ExitStack() as ctx:
        _emit(ctx, tc, io, bcoef, n)
    nc.compile()
    _CACHE[key] = nc
    return nc


def _prep(q, r, hop, W0, A, Bm, v, b):
    q = np.asarray(q, dtype=np.float32)
    r = np.asarray(r, dtype=np.float32)
    hop = np.asarray(hop)
    if hop.dtype != np.int32:
        hop = hop.astype(np.int32)
    W0 = np.asarray(W0, dtype=np.float32)
    A = np.asarray(A, dtype=np.float32)
    Bm = np.asarray(Bm, dtype=np.float32)
    v = np.asarray(v, dtype=np.float32)
    b = np.asarray(b, dtype=np.float64)

    wp = (W0[0] + np.diag(v)).astype(np.float16)
    ac = A.transpose(1, 0, 2).reshape(D, (L + 1) * RHO).astype(np.float16)
    bc = Bm.transpose(1, 0, 2).reshape(D, (L + 1) * RHO).astype(np.float16)
    idm = np.eye(P, dtype=np.float16)
    sel = np.zeros((P, 63), dtype=np.float16)
    sel[:, 31] = 1.0
    gid = (np.arange((L + 1) * RHO) // RHO).reshape(-1, 1).astype(np.float16)
    bcoef = np.polyfit(np.arange(L + 1, dtype=np.float64), b, L)

    consts = dict(wp=wp, ac=ac, bc=bc, idm=idm, sel=sel, gid=gid)
    in_maps = []
    for c in range(NCORES):
        sl = slice(c * N, (c + 1) * N)
        in_maps.append(
            dict(q=q[sl], r=r[sl], hop=hop[sl], **consts)
        )
    return in_maps, bcoef


def _run(inputs, trace=False, tmpdir=None):
    in_maps, bcoef = _prep(**inputs)
    nc = _build(bcoef)
    res = run_bass_kernel_spmd(
        nc, in_maps, list(range(NCORES)), trace=trace, tmpdir=tmpdir
    )
    out = np.concatenate([np.asarray(res.results[c]["o"]) for c in range(NCORES)])
    return out, res


def kernel(**inputs):
    out, _ = _run(inputs)
    return out
